# revision 5
# baseline (speedup 1.0000x reference)
"""Trainium2 Bass kernel for the CenterNet-style detection head + NMS compaction.

Sharding: 8 cores = 2 images x 4 row-bands (20 rows each).  Each core
uploads only its x slice (24 rows incl. conv+pool halo, ~500KB) and runs:
conv1 3x3 for the 3 heads (hm head in fp32 so the local-maxima equality
pattern is bit-stable; wh/reg heads in bf16 -- bbox tolerance is loose),
conv2, the 3x3 stride-1 max-pool local-maxima mask for all 80 classes of
its band (pre-sigmoid padded with -1e30; equality is invariant under the
monotone sigmoid), sigmoid scores, and the bbox decode replicating the
reference's fp32 op order.

Device outputs are tiny (vs. dense 128k-candidate rows): masked sigmoid
scores (mask * sigmoid, bf16, [80 classes, 1600 px]) and pixel-major bbox
[cx,cy,w,h] (f32).  Scores are strictly positive, so nonzero == maxima;
the host reconstructs the (2, 512000, 85) output by flatnonzero in
class-major scan order (the reference's stable compaction order), fills
bbox/score columns, and scatters the one-hot class columns.

Band edges: the pool's out-of-image rows must be -inf, not conv output of
the zero-padded halo, so a per-core `rowclip` bias (0 or -1e30) is added
to the two halo rows of the padded heatmap -- keeps the program SPMD-
identical across cores with all differences in data.
"""

import numpy as np

NB, CH, NY, NX, NCLS = 2, 64, 80, 80, 80
G = 4              # row-bands (cores per image)
RB = NY // G       # rows per band = 20
XR = RB + 4        # x rows per core incl halo = 24
HR = RB + 2        # hm rows per core incl pool halo = 22
PW = NX + 2        # padded width 82
NPB = RB * NX      # band pixels = 1600
NPH = HR * NX      # hm pixels incl halo = 1760
NTL = 5            # conv tiles per core (5,5,5,5,2 rows)
BT = 13            # pixel-major 128-wide tiles for whreg (12*128 + 64)

_CACHE = {}


def _build_program():
    import concourse.bacc as bacc
    import concourse.mybir as mybir
    from concourse.ap import AP
    from concourse.tile import TileContext
    from contextlib import ExitStack

    f32 = mybir.dt.float32
    bf16 = mybir.dt.bfloat16
    AF = mybir.ActivationFunctionType
    OP = mybir.AluOpType

    def v(base_ap, off, dims):
        # dims[0] = [1, npart] placeholder; real partition step is the row
        # stride of the underlying tensor (offset convention: p*stride + f)
        rs = base_ap.ap[0][0]
        return AP(base_ap.tensor, base_ap.offset + off,
                  [[rs, dims[0][1]]] + [list(d) for d in dims[1:]])

    nc = bacc.Bacc("TRN2", target_bir_lowering=False, debug=False, num_devices=8)

    xt_d = nc.dram_tensor("xt", [64, XR * PW], f32, kind="ExternalInput").ap()
    w1hm_d = nc.dram_tensor("w1hm", [64, 576], f32, kind="ExternalInput").ap()
    w1wr_d = nc.dram_tensor("w1wr", [64, 1152], bf16, kind="ExternalInput").ap()
    misc_d = nc.dram_tensor("misc", [128, 176], f32, kind="ExternalInput").ap()

    msig_d = nc.dram_tensor("msig", [NCLS, NPB], bf16, kind="ExternalOutput").ap()
    bbox_d = nc.dram_tensor("bbox", [128, 4 * BT], f32, kind="ExternalOutput").ap()

    # misc layout (cols): 0:80 w2hm(rows 0:64) | 80 b2hm(rows 0:80) |
    # 81:84 b1 hm/wh/reg (rows 0:64) | 84:88 w2blk | 88:140 bwr |
    # 140:166 g1 | 166:168 rowclip(rows 0:80)

    with TileContext(nc) as tc, ExitStack() as ex:
        consts = ex.enter_context(tc.tile_pool(name="consts", bufs=1))
        w1hm = consts.tile([64, 576], f32, tag="w1hm")
        nc.sync.dma_start(out=w1hm[:, :], in_=w1hm_d)
        w1wr = consts.tile([64, 1152], bf16, tag="w1wr")
        nc.sync.dma_start(out=w1wr[:, :], in_=w1wr_d)
        misc = consts.tile([128, 176], f32, tag="misc")
        nc.sync.dma_start(out=misc[:, :], in_=misc_d)

        work = ex.enter_context(tc.tile_pool(name="work", bufs=1))
        xt = work.tile([64, XR * PW], f32, tag="xt")
        nc.sync.dma_start(out=xt[:, :], in_=xt_d)
        xtb = work.tile([64, XR * PW], bf16, tag="xtb")
        nc.vector.tensor_copy(xtb[:, :], xt[:, :])
        w2blkb = work.tile([128, 4], bf16, tag="w2blkb")
        nc.vector.tensor_copy(w2blkb[:, :], misc[:, 84:88])

        y1hm = work.tile([64, NPH], f32, tag="y1hm")
        y1wr = work.tile([128, NPH], bf16, tag="y1wr")  # wh 0:64, reg 64:128

        rows_of = [(5 * t, min(5, HR - 5 * t)) for t in range(NTL)]

        # ---------- conv1: 3x3 stride-1, 64->64, relu, 3 heads ----------
        with tc.tile_pool(name="ps1", bufs=4, space="PSUM") as ps1:
            for head in range(3):
                for r0, nr in rows_of:
                    npx = nr * NX
                    ps = ps1.tile([64, npx], f32, tag=f"c1_{nr}")
                    k = 0
                    for ky in range(3):
                        for kx in range(3):
                            if head == 0:
                                wsl = w1hm[:, (3 * ky + kx) * 64:
                                           (3 * ky + kx + 1) * 64]
                                rhs = v(xt[:, :], (r0 + ky) * PW + kx,
                                        [[1, 64], [PW, nr], [1, NX]])
                            else:
                                c0 = ((head - 1) * 9 + 3 * ky + kx) * 64
                                wsl = w1wr[:, c0:c0 + 64]
                                rhs = v(xtb[:, :], (r0 + ky) * PW + kx,
                                        [[1, 64], [PW, nr], [1, NX]])
                            nc.tensor.matmul(ps[:, :], wsl, rhs,
                                             start=(k == 0), stop=(k == 8))
                            k += 1
                    if head == 0:
                        dst = y1hm[:, r0 * NX:r0 * NX + npx]
                    elif head == 1:
                        dst = y1wr[0:64, r0 * NX:r0 * NX + npx]
                    else:
                        dst = y1wr[64:128, r0 * NX:r0 * NX + npx]
                    nc.scalar.activation(dst, ps[:, :], AF.Relu,
                                         bias=misc[0:64, 81 + head:82 + head])

        # ---------- conv2 hm (1x1, 64->80) into padded tile ----------
        pb = ex.enter_context(tc.tile_pool(name="pb", bufs=1))
        hmpad = pb.tile([NCLS, HR * PW], f32, tag="hmpad")
        hp = hmpad[:, :]
        nc.vector.memset(hp, -1.0e30)
        with tc.tile_pool(name="ps2", bufs=2, space="PSUM") as ps2p:
            for r0, nr in rows_of:
                npx = nr * NX
                ps = ps2p.tile([NCLS, npx], f32, tag=f"c2_{nr}")
                nc.tensor.matmul(ps[:, :], misc[0:64, 0:80],
                                 y1hm[:, r0 * NX:r0 * NX + npx],
                                 start=True, stop=True)
                inner = v(hp, (r0 * PW) + 1, [[1, NCLS], [PW, nr], [1, NX]])
                nc.scalar.add(inner, ps[:, :], misc[0:NCLS, 80:81])
        # clip halo rows to -1e30 where out-of-image (rowclip is 0 or -1e30)
        row0 = v(hp, 0, [[1, NCLS], [1, PW]])
        nc.scalar.add(row0, row0, misc[0:NCLS, 166:167])
        rowL = v(hp, (HR - 1) * PW, [[1, NCLS], [1, PW]])
        nc.scalar.add(rowL, rowL, misc[0:NCLS, 167:168])

        # ---------- 3x3 max pool (separable), mask, masked sigmoid ----------
        rowm = pb.tile([NCLS, HR * NX], f32, tag="rowm")
        rm = rowm[:, :]
        s_in = lambda off: v(hp, off, [[1, NCLS], [PW, HR], [1, NX]])
        rm_full = v(rm, 0, [[1, NCLS], [NX, HR], [1, NX]])
        nc.vector.tensor_tensor(rm_full, s_in(0), s_in(1), op=OP.max)
        nc.vector.tensor_tensor(rm_full, rm_full, s_in(2), op=OP.max)
        hmax = pb.tile([NCLS, NPB], f32, tag="hmax")
        hx = hmax[:, :]
        r_sh = lambda off: v(rm, off, [[1, NCLS], [NX, RB], [1, NX]])
        nc.vector.tensor_tensor(hx, r_sh(0), r_sh(NX), op=OP.max)
        nc.vector.tensor_tensor(hx, hx, r_sh(2 * NX), op=OP.max)
        hm_inner = v(hp, PW + 1, [[1, NCLS], [PW, RB], [1, NX]])
        maskf = pb.tile([NCLS, NPB], f32, tag="maskf")
        nc.vector.tensor_tensor(maskf[:, :], hx, hm_inner, op=OP.is_equal)

        # sigma = 1/(1+exp(-hm)); masked scores to bf16 (sig>0 so
        # nonzero == maxima on the host side)
        sig = pb.tile([NCLS, NPB], f32, tag="sig")
        nc.scalar.activation(sig[:, :], hm_inner, AF.Exp, scale=-1.0)
        nc.vector.tensor_scalar_add(sig[:, :], sig[:, :], 1.0)
        nc.vector.reciprocal(sig[:, :], sig[:, :])
        msb = pb.tile([NCLS, NPB], bf16, tag="msb")
        nc.vector.tensor_tensor(msb[:, :], sig[:, :], maskf[:, :], op=OP.mult)
        nc.sync.dma_start(out=msig_d, in_=msb[:, :])

        # ---------- conv2 wh/reg (pixel-major via block-diag rhs), decode ----
        with tc.tile_pool(name="psw", bufs=1, space="PSUM") as pswp:
            psw = pswp.tile([128, 4 * BT], f32)
            for t in range(BT):
                n = min(128, NPB - t * 128)
                nc.tensor.matmul(psw[0:n, 4 * t:4 * t + 4],
                                 y1wr[:, NX + 128 * t:NX + 128 * t + n],
                                 w2blkb[:, :], start=True, stop=True)
            tmp = pb.tile([128, 4 * BT], f32, tag="tmp")
            nc.vector.tensor_tensor(tmp[:, :], psw[:, :], misc[:, 88:140],
                                    op=OP.add)
        nc.vector.tensor_scalar_max(tmp[:, :], tmp[:, :], 0.0)
        # ctr = g1 + reg; half = wh*0.5; a4 = (ctr-half)*4; b4 = (ctr+half)*4
        # cxy = (a4+b4)*0.5; bwh = b4-a4   (reference fp32 op order)
        ctr = pb.tile([128, 2 * BT], f32, tag="ctr")
        half = pb.tile([128, 2 * BT], f32, tag="half")
        a4 = pb.tile([128, 2 * BT], f32, tag="a4")
        b4 = pb.tile([128, 2 * BT], f32, tag="b4")
        bboxw = pb.tile([128, 4 * BT], f32, tag="bboxw")
        dBTx2 = [[1, 128], [4, BT], [1, 2]]
        tmp_wh = v(tmp[:, :], 0, dBTx2)
        tmp_reg = v(tmp[:, :], 2, dBTx2)
        nc.vector.tensor_tensor(ctr[:, :], tmp_reg, misc[:, 140:166], op=OP.add)
        nc.vector.tensor_scalar_mul(half[:, :], tmp_wh, 0.5)
        nc.vector.tensor_tensor(a4[:, :], ctr[:, :], half[:, :], op=OP.subtract)
        nc.vector.tensor_scalar_mul(a4[:, :], a4[:, :], 4.0)
        nc.vector.tensor_tensor(b4[:, :], ctr[:, :], half[:, :], op=OP.add)
        nc.vector.tensor_scalar_mul(b4[:, :], b4[:, :], 4.0)
        bb_cxy = v(bboxw[:, :], 0, dBTx2)
        bb_wh = v(bboxw[:, :], 2, dBTx2)
        nc.vector.tensor_tensor(bb_cxy, a4[:, :], b4[:, :], op=OP.add)
        nc.vector.tensor_scalar_mul(bb_cxy, bb_cxy, 0.5)
        nc.vector.tensor_tensor(bb_wh, b4[:, :], a4[:, :], op=OP.subtract)
        nc.sync.dma_start(out=bbox_d, in_=bboxw[:, :])

    nc.compile()
    return nc


def _prep_inputs(x, offsets, hm_w1, hm_b1, hm_w2, hm_b2,
                 wh_w1, wh_b1, wh_w2, wh_b2, reg_w1, reg_b1, reg_w2, reg_b2):
    import ml_dtypes
    f32 = np.float32
    bf16 = np.dtype(ml_dtypes.bfloat16)
    x = np.asarray(x, f32)

    def t_(w):  # (O,I,ky,kx) -> [I,O,ky,kx]
        return np.transpose(np.asarray(w, f32), (1, 0, 2, 3))

    w1hm = np.zeros((64, 576), f32)
    whm = t_(hm_w1)
    for ky in range(3):
        for kx in range(3):
            w1hm[:, (3 * ky + kx) * 64:(3 * ky + kx + 1) * 64] = whm[:, :, ky, kx]
    w1wr = np.zeros((64, 1152), f32)
    for head, w in enumerate([t_(wh_w1), t_(reg_w1)]):
        for ky in range(3):
            for kx in range(3):
                c0 = (head * 9 + 3 * ky + kx) * 64
                w1wr[:, c0:c0 + 64] = w[:, :, ky, kx]
    w1wr = w1wr.astype(bf16)

    misc0 = np.zeros((128, 176), f32)
    misc0[0:64, 0:80] = np.asarray(hm_w2, f32)[:, :, 0, 0].T
    misc0[0:NCLS, 80] = np.asarray(hm_b2, f32)
    misc0[0:64, 81] = np.asarray(hm_b1, f32)
    misc0[0:64, 82] = np.asarray(wh_b1, f32)
    misc0[0:64, 83] = np.asarray(reg_b1, f32)
    misc0[0:64, 84:86] = np.asarray(wh_w2, f32)[:, :, 0, 0].T
    misc0[64:128, 86:88] = np.asarray(reg_w2, f32)[:, :, 0, 0].T
    bwr4 = np.array([wh_b2[0], wh_b2[1], reg_b2[0], reg_b2[1]], f32)
    misc0[:, 88:140] = np.tile(bwr4, BT)[None, :]

    p = np.arange(128 * BT)
    px = (p % NX).astype(f32)          # x coord of band pixel p
    py = (p // NX).astype(f32)         # local y coord
    pvalid = p < NPB

    in_maps = []
    for core in range(8):
        b, g = divmod(core, G)
        y0 = RB * g
        xp = np.zeros((64, XR, PW), f32)
        lo, hi = y0 - 2, y0 + RB + 2
        a, bb = max(0, lo), min(NY, hi)
        xp[:, a - lo:bb - lo, 1:NX + 1] = x[b, :, a:bb, :]

        misc = misc0.copy()
        off2 = np.asarray(offsets, f32)[b, 1:3] * f32(2.0)
        gx = (px + off2[0]) * pvalid
        gy = (py + f32(y0) + off2[1]) * pvalid
        # g1 pixel-major: partition i, tile t -> pixel p = 128t + i
        g1 = np.stack([gx, gy], axis=-1).reshape(BT, 128, 2)
        misc[:, 140:166] = g1.transpose(1, 0, 2).reshape(128, 2 * BT)
        misc[0:NCLS, 166] = 0.0 if g > 0 else -1.0e30
        misc[0:NCLS, 167] = 0.0 if g < G - 1 else -1.0e30

        in_maps.append({
            "xt": np.ascontiguousarray(xp.reshape(64, XR * PW)),
            "w1hm": w1hm, "w1wr": w1wr, "misc": misc,
        })
    return in_maps


def _get_nc():
    if "nc" not in _CACHE:
        _CACHE["nc"] = _build_program()
    return _CACHE["nc"]


def run_cores(in_maps, trace=False):
    from concourse import bass_utils
    nc = _get_nc()
    return bass_utils.run_bass_kernel_spmd(nc, in_maps, list(range(8)),
                                           trace=trace)


def assemble(results):
    out = np.zeros((NB, NCLS * NY * NX, 5 + NCLS), np.float32)
    for b in range(NB):
        # [80, 4, 1600] -> class-major image-flat [80, 6400]
        msig = np.stack([np.asarray(results[b * G + g]["msig"])
                         for g in range(G)], axis=1)
        msig = msig.astype(np.float32).reshape(NCLS, NY * NX)
        bbox = np.concatenate([
            np.asarray(results[b * G + g]["bbox"])
            .reshape(128, BT, 4).transpose(1, 0, 2).reshape(128 * BT, 4)[:NPB]
            for g in range(G)], axis=0)          # [6400, 4]
        flat = msig.reshape(-1)
        idx = np.flatnonzero(flat)
        n = idx.size
        out[b, :n, 0:4] = bbox[idx % (NY * NX)]
        out[b, :n, 4] = flat[idx]
        out[b, np.arange(n), 5 + idx // (NY * NX)] = 1.0
    return out


def kernel(**inputs):
    in_maps = _prep_inputs(**{k: np.asarray(v) for k, v in inputs.items()})
    res = run_cores(in_maps)
    return assemble(res.results)
